# revision 43
# baseline (speedup 1.0000x reference)
"""Multi-head attention (B=2, S=2048, D=1024, H=16, Dh=64) on 8 TRN2 cores.

Sharding: data-parallel over batch (2) x tensor-parallel over heads (16 -> 4
groups of 4). Core c handles batch c//4, heads [4*(c%4), 4*(c%4)+4).
Each core computes its partial output projection (Wo column slice); the host
sums the 4 per-core partials per batch (the "all-reduce") and adds bo.

All-fp16 data path (fp8/DoubleRow was tried and is numerically dead here:
quantization noise on scores/weights/values does NOT average down — the
attention output is a weighted mean whose magnitude shrinks as fast as the
noise, so fp8 anywhere in the value path lands at ~3-7% output error vs the
2e-2 gate; fp16 gives ~9e-4).

The kernel is PE-bound (fp16 matmul floor ~401k PE cycles = 167us at the
2.4GHz max p-state vs ACT's 133us exp stream). The TRN2 p-state model
punishes recurring PE dependency stalls (they reset the clock ramp and lock
the PE at 1.2GHz), and the cost model serializes all DMA transfers on one
track (~2.9us per 0.5MB s-chunk), which shapes the whole schedule:

  - x streams in s-chunks [P, KD, 512] in exact consumption order on one
    queue; every projection job is runnable right after one chunk lands.
  - V is projected directly in natural [s, j] layout (x chunk stationary,
    Wv moving): no PE transposes of V^T.
  - attn@V(i) runs one full task LATER (task-lag): it reads only buffered
    exp tiles + V, so the PE never stalls on the live exp stream. Only the
    last task consumes its own exps lag-1, and it compresses attn@V(6)
    into its first half so only norm(7) + the qh1 output projection remain
    after the final exp.
  - Filler jobs (jb1/late projections, V s-blocks, qh0 output projection)
    are placed at hand-tuned (task, kb) slots to keep the PE dense.
"""

import numpy as np
from contextlib import ExitStack

import concourse.bass as bass
from concourse import bacc
import concourse.mybir as mybir
import concourse.tile as tile

F32 = mybir.dt.float32
F32R = mybir.dt.float32r
F16 = mybir.dt.float16
AF = mybir.ActivationFunctionType

B = 2
S = 2048
D = 1024
H = 16
DH = 64
NCORES = 8
HL = 4          # heads per core
J = HL * DH     # 256 local projection width
P = 128
KD = D // P     # 8 d-chunks
KB = S // P     # 16 k-blocks
QH = S // 1024  # 2 q-halves of 1024
EB = D // P     # 8 e-blocks
NS = 4          # s-chunks of 512


def build_nc():
    nc = bacc.Bacc()

    xq = nc.dram_tensor("xq", [P, NS, KD, 512], F16, kind="ExternalInput")
    xk = nc.dram_tensor("xk", [P, NS, KD, 512], F16, kind="ExternalInput")
    xv = nc.dram_tensor("xv", [P, NS, KD, 512], F16, kind="ExternalInput")
    wq = nc.dram_tensor("wq", [P, KD, J], F16, kind="ExternalInput")
    wk = nc.dram_tensor("wk", [P, KD, J], F16, kind="ExternalInput")
    wv = nc.dram_tensor("wv", [P, KD, J], F16, kind="ExternalInput")
    wo = nc.dram_tensor("wo", [P, 2, D], F16, kind="ExternalInput")
    out_t = nc.dram_tensor("out_t", [EB, P, S], F16, kind="ExternalOutput")

    with tile.TileContext(nc) as tc, ExitStack() as st:
        const = st.enter_context(tc.tile_pool(name="const", bufs=1))
        persist = st.enter_context(tc.tile_pool(name="persist", bufs=1))
        xpool = st.enter_context(tc.tile_pool(name="xstream", bufs=12))

        wq_sb = const.tile([P, KD, J], F16, tag="wq")
        wk_sb = const.tile([P, KD, J], F16, tag="wk")
        wv_sb = const.tile([P, KD, J], F16, tag="wv")
        wo_sb = const.tile([P, 2, D], F16, tag="wo")

        qt_sb = persist.tile([P, 2, S], F16, tag="qt")   # Q_T [256, 2048]
        kt_sb = persist.tile([P, 2, S], F16, tag="kt")   # K_T
        # V natural layout + per-head ones col: [s_part, kb, h, 65]
        v_sb = persist.tile([P, KB, HL, DH + 1], F16, tag="v")
        ao_sb = persist.tile([P, 2, S], F16, tag="ao")   # normalized attn out ^T

        ones64 = const.tile([1, DH], F32R, tag="ones64")

        # --- DMA: one serial stream; order = consumption order -----------
        xq_t = [xpool.tile([P, KD, 512], F16, tag="xc", name=f"xq{c}")
                for c in range(NS)]
        xk_t = [xpool.tile([P, KD, 512], F16, tag="xc", name=f"xk{c}")
                for c in range(NS)]
        xv_t = [xpool.tile([P, KD, 512], F16, tag="xc", name=f"xv{c}")
                for c in range(NS)]
        ones16 = nc.inline_tensor(np.ones((P, KB * HL), np.float16),
                                  name="ones16")
        ones_f32 = nc.inline_tensor(np.ones((1, DH), np.float32), name="ones_f")
        dma_order = [
            (wk_sb, wk, None),
            (xk_t[0], xk, 0), (wq_sb, wq, None),
            (xq_t[0], xq, 0), (xq_t[1], xq, 1),
            (xk_t[1], xk, 1), (xk_t[2], xk, 2), (xk_t[3], xk, 3),
            (wv_sb, wv, None),
            (xv_t[0], xv, 0), (xq_t[2], xq, 2), (xv_t[1], xv, 1),
            (xq_t[3], xq, 3), (xv_t[2], xv, 2), (xv_t[3], xv, 3),
        ]
        for dst, srcd, sch in dma_order:
            if sch is None:
                nc.sync.dma_start(out=dst[:], in_=srcd[:])
            else:
                nc.sync.dma_start(out=dst[:], in_=srcd[:, sch])
        nc.sync.dma_start(out=v_sb[:, :, :, DH], in_=ones16.ap())
        nc.sync.dma_start(out=ones64[:], in_=ones_f32.ap().bitcast(F32R))
        nc.sync.dma_start(out=wo_sb[:], in_=wo[:])

        # --- attention pipeline with interleaved filler work --------------
        with tc.tile_pool(name="psc", bufs=2, space="PSUM") as psc, tc.tile_pool(
            name="poacc", bufs=2, space="PSUM"
        ) as poacc, tc.tile_pool(name="expp", bufs=17) as expp, tc.tile_pool(
            name="npool", bufs=2
        ) as npool, tc.tile_pool(name="ostage", bufs=4) as opool:

            def proj_job(wsb, xts, dst, sch, jb, nm):
                def f():
                    pp = psc.tile([P, 512], F32, tag="sc",
                                  name=f"pj{nm}{sch}{jb}")
                    for c in range(KD):
                        nc.tensor.matmul(
                            pp[:, :512],
                            wsb[:, c, jb * P:(jb + 1) * P],
                            xts[sch][:, c, :],
                            start=(c == 0),
                            stop=(c == KD - 1),
                        )
                    nc.vector.tensor_copy(
                        dst[:, jb, sch * 512:(sch + 1) * 512], pp[:])
                return f

            def vjob(sb):
                def f():
                    vp = psc.tile([P, J], F32, tag="sc", name=f"vp{sb}")
                    for c in range(KD):
                        nc.tensor.matmul(
                            vp[:, :J],
                            xv_t[sb // 4][:, c, (sb % 4) * P:(sb % 4 + 1) * P],
                            wv_sb[:, c, :],
                            start=(c == 0),
                            stop=(c == KD - 1),
                        )
                    nc.vector.tensor_copy(v_sb[:, sb, :, 0:DH], vp[:])
                return f

            def oproj_eb(qh, eb, evac="dve", tailpool=False,
                         split_dma=False):
                def f():
                    q0 = qh * 1024
                    ob = opool.tile([P, 1024], F16, tag="ob",
                                    name=f"ob{(qh * EB + eb) % 4}")
                    for stl in range(2):
                        s0 = q0 + stl * 512
                        pool = poacc if (tailpool and stl == 1) else psc
                        tg = "oacc" if (tailpool and stl == 1) else "sc"
                        po = pool.tile([P, 512], F32, tag=tg,
                                       name=f"po_{qh}_{eb}_{stl}")
                        for jbx in range(2):
                            nc.tensor.matmul(
                                po[:, :512],
                                wo_sb[:, jbx, eb * P:(eb + 1) * P],
                                ao_sb[:, jbx, s0:s0 + 512],
                                start=(jbx == 0),
                                stop=(jbx == 1),
                            )
                        d = ob[:, stl * 512:(stl + 1) * 512]
                        if evac == "act" or (evac == "mix" and stl == 0):
                            nc.scalar.copy(d, po[:, :512])
                        else:
                            nc.vector.tensor_copy(d, po[:, :512])
                        if split_dma:
                            nc.sync.dma_start(out=out_t[eb][:, s0:s0 + 512],
                                              in_=d)
                    if not split_dma:
                        nc.sync.dma_start(out=out_t[eb][:, q0:q0 + 1024],
                                          in_=ob[:])
                return f

            # filler schedule: fillers[(task, kb)] run inside the kb loop
            # after scores/exp/attn@V; positions tuned against the model
            fillers = {}

            def put(i, kb, fn):
                fillers.setdefault((i, kb), []).append(fn)

            put(0, 3, proj_job(wk_sb, xk_t, kt_sb, 1, 0, "k"))
            put(0, 7, proj_job(wk_sb, xk_t, kt_sb, 2, 0, "k"))
            put(0, 10, proj_job(wk_sb, xk_t, kt_sb, 3, 0, "k"))
            put(0, 0, proj_job(wk_sb, xk_t, kt_sb, 0, 1, "k"))
            put(0, 1, proj_job(wq_sb, xq_t, qt_sb, 0, 1, "q"))
            put(0, 4, proj_job(wq_sb, xq_t, qt_sb, 1, 1, "q"))
            put(0, 5, proj_job(wk_sb, xk_t, kt_sb, 1, 1, "k"))
            put(0, 8, proj_job(wk_sb, xk_t, kt_sb, 2, 1, "k"))
            put(0, 12, proj_job(wk_sb, xk_t, kt_sb, 3, 1, "k"))
            for n in range(3):
                put(0, 13 + n, vjob(n))
            put(1, 0, vjob(3))
            for n in range(4):
                put(1, 1 + n, vjob(4 + n))
            for n in range(4):
                put(1, 5 + n, vjob(8 + n))
            for n in range(4):
                put(1, 9 + n, vjob(12 + n))
            put(1, 13, proj_job(wq_sb, xq_t, qt_sb, 2, 0, "q"))
            put(1, 14, proj_job(wq_sb, xq_t, qt_sb, 3, 0, "q"))
            put(1, 15, proj_job(wq_sb, xq_t, qt_sb, 2, 1, "q"))
            put(2, 0, proj_job(wq_sb, xq_t, qt_sb, 3, 1, "q"))
            # ao(qh0) is complete only after norm(3), which lands at the
            # END of task 4 (task-lag) — oproj(0) fillers start in task 5
            opos = [(5, 2), (5, 7), (5, 12), (6, 2), (6, 7), (6, 12),
                    (7, 2), (7, 7)]
            for eb in range(EB):
                put(*opos[eb], oproj_eb(0, eb))

            def normalize(h, q0, w, oacc, recip, nm):
                def f():
                    jbn = h // 2
                    off = DH * (h % 2)
                    bc = psc.tile([DH, w], F32, tag="sc", name=f"bc{nm}")
                    for n in range(w // 512):
                        nc.tensor.matmul(
                            bc[:, n * 512:(n + 1) * 512],
                            ones64[:],
                            recip[:, n * 512:(n + 1) * 512],
                            start=True,
                            stop=True,
                        )
                    bcast = npool.tile([DH, w], F16, tag="bcast",
                                       name=f"bst{nm}")
                    nc.vector.tensor_copy(bcast[:], bc[:])
                    nc.vector.tensor_mul(
                        ao_sb[off:off + DH, jbn, q0:q0 + w],
                        oacc[0:DH, :],
                        bcast[:],
                    )
                return f

            # pre-task projections: first scores need Q sch0-1 + K sch0 (jb0)
            proj_job(wk_sb, xk_t, kt_sb, 0, 0, "k")()
            proj_job(wq_sb, xq_t, qt_sb, 0, 0, "q")()
            proj_job(wq_sb, xq_t, qt_sb, 1, 0, "q")()

            # attn@V(i) runs one full task later (task-lag): it reads only
            # buffered exps + V, so the PE never stalls on the live exp
            # stream (a per-kb dependency stall would reset the clock ramp
            # and lock the PE at 1.2GHz). The last task additionally
            # consumes its own exps lag-1 and compresses attn@V(6) into
            # its first half so the tail is just norm(7) + the qh1 oproj.
            tasks = [(qh, hh) for qh in range(QH) for hh in range(HL)]
            NT = len(tasks)

            def attn_v(oacc, hh, ex, kb):
                for n in range(2):
                    nc.tensor.matmul(
                        oacc[:, n * 512:(n + 1) * 512],
                        v_sb[:, kb, hh, :],
                        ex[kb][:, n * 512:(n + 1) * 512],
                        start=(kb == 0),
                        stop=(kb == KB - 1),
                    )

            prev = None  # (qh, h, ex) whose attn@V runs this iter
            exn = 0
            for i in range(NT):
                qh, h = tasks[i]
                q0 = qh * 1024
                jb = h // 2
                off = DH * (h % 2)
                cur = (qh, h, {})
                oacc = poacc.tile([DH + 1, 1024], F32, tag="oacc",
                                  name=f"oacc{i % 2}") if prev else None
                oacc7 = poacc.tile([DH + 1, 1024], F32, tag="oacc",
                                   name="oacc7") if i == NT - 1 else None
                for kb in range(KB):
                    sc = psc.tile([P, 1024], F32, tag="sc", name=f"sc{kb % 2}")
                    for n in range(2):
                        nc.tensor.matmul(
                            sc[:, n * 512:(n + 1) * 512],
                            kt_sb[off:off + DH, jb, kb * P:(kb + 1) * P],
                            qt_sb[off:off + DH, jb,
                                  q0 + n * 512:q0 + (n + 1) * 512],
                            start=True,
                            stop=True,
                        )
                    cur[2][kb] = expp.tile([P, 1024], F16, tag="ex",
                                           name=f"ex{exn % 17}")
                    exn += 1
                    nc.scalar.activation(cur[2][kb][:], sc[:], AF.Exp)
                    if prev is not None:
                        if oacc7 is not None:
                            if kb < KB // 2:
                                attn_v(oacc, prev[1], prev[2], 2 * kb)
                                attn_v(oacc, prev[1], prev[2], 2 * kb + 1)
                        else:
                            attn_v(oacc, prev[1], prev[2], kb)
                    if oacc7 is not None and kb >= 1:
                        attn_v(oacc7, h, cur[2], kb - 1)
                    if oacc7 is not None and kb == KB // 2:
                        recip6 = npool.tile([1, 1024], F32R, tag="recip",
                                            name="recip6")
                        with nc.allow_low_precision(reason="softmax denom"):
                            nc.vector.reciprocal(recip6[:],
                                                 oacc[DH:DH + 1, :])
                    if oacc7 is not None and kb == KB // 2 + 2:
                        normalize(prev[1], prev[0] * 1024, 1024, oacc,
                                  recip6, "n6")()
                    for fn in fillers.pop((i, kb), ()):
                        fn()
                if prev is not None and oacc7 is None:
                    recip = npool.tile([1, 1024], F32R, tag="recip",
                                       name=f"recip{i % 2}")
                    with nc.allow_low_precision(reason="fp32r softmax denom"):
                        nc.vector.reciprocal(recip[:], oacc[DH:DH + 1, :])
                    normalize(prev[1], prev[0] * 1024, 1024, oacc, recip,
                              f"n{i % 2}")()
                if oacc7 is not None:
                    attn_v(oacc7, h, cur[2], KB - 1)
                    recip7 = npool.tile([1, 1024], F32R, tag="recip",
                                        name="recip7")
                    with nc.allow_low_precision(reason="fp32r softmax denom"):
                        nc.vector.reciprocal(recip7[:], oacc7[DH:DH + 1, :])
                    normalize(h, q0, 1024, oacc7, recip7, "nd")()
                    prev = None
                else:
                    prev = cur

            # tail: qh1 output projection, po tiles spread over both pools
            for eb in range(EB):
                oproj_eb(1, eb, evac="mix", tailpool=True)()

    nc.finalize()
    return nc


_NC_CACHE = None


def _get_nc():
    global _NC_CACHE
    if _NC_CACHE is None:
        _NC_CACHE = build_nc()
    return _NC_CACHE


def make_in_maps(query, key, value, Wq, Wk, Wv, Wo):
    """Build the 8 per-core input dicts from the full tensors."""
    query = np.asarray(query, np.float32)
    key = np.asarray(key, np.float32)
    value = np.asarray(value, np.float32)
    Wq = np.asarray(Wq, np.float32)
    Wk = np.asarray(Wk, np.float32)
    Wv = np.asarray(Wv, np.float32)
    Wo = np.asarray(Wo, np.float32)

    def pmajor(a2d, inner):  # [Drows, inner] -> [P, Drows//P, inner]
        return np.ascontiguousarray(
            a2d.reshape(KD, P, inner).transpose(1, 0, 2)
        )

    def schunk(a2d):  # X^T [D, S] -> [P, NS, KD, 512]
        return np.ascontiguousarray(
            a2d.reshape(KD, P, NS, 512).transpose(1, 2, 0, 3)
        )

    scale = np.float32(1.0 / np.sqrt(DH))
    xs = {}
    for b in range(B):
        xs[b] = {
            "xq": schunk(np.ascontiguousarray(query[b].T)).astype(np.float16),
            "xk": schunk(np.ascontiguousarray(key[b].T)).astype(np.float16),
            "xv": schunk(np.ascontiguousarray(value[b].T)).astype(np.float16),
        }
    ws = {}
    for hg in range(4):
        sl = slice(hg * J, (hg + 1) * J)
        wo_t = np.ascontiguousarray(Wo[:, sl].T)  # [256, 1024]
        ws[hg] = {
            "wq": pmajor(np.ascontiguousarray(Wq[sl].T * scale), J).astype(
                np.float16),
            "wk": pmajor(np.ascontiguousarray(Wk[sl].T), J).astype(np.float16),
            "wv": pmajor(np.ascontiguousarray(Wv[sl].T), J).astype(np.float16),
            "wo": np.ascontiguousarray(
                wo_t.reshape(2, P, D).transpose(1, 0, 2)
            ).astype(np.float16),
        }
    in_maps = []
    for c in range(NCORES):
        b, hg = c // 4, c % 4
        m = {}
        m.update(xs[b])
        m.update(ws[hg])
        in_maps.append(m)
    return in_maps


def assemble(results, bo):
    """Sum the 4 per-core partials per batch, add bo."""
    bo = np.asarray(bo, np.float32)
    out = np.zeros((B, S, D), np.float32)
    for c in range(NCORES):
        b = c // 4
        part = results[c]["out_t"].astype(np.float32).reshape(D, S).T
        out[b] += part
    out += bo[None, None, :]
    return out


def kernel(query, key, value, Wq, Wk, Wv, Wo, bo):
    import os
    import time

    os.environ.setdefault("NEURON_RT_RESET_CORES", "1")
    from concourse.bass_utils import run_bass_kernel_spmd

    nc = _get_nc()
    in_maps = make_in_maps(query, key, value, Wq, Wk, Wv, Wo)
    last_exc = None
    for attempt in range(3):
        try:
            res = run_bass_kernel_spmd(nc, in_maps, list(range(NCORES)))
            return assemble(res.results, bo)
        except Exception as e:  # transient NRT_EXEC_UNIT_UNRECOVERABLE etc.
            last_exc = e
            time.sleep(2.0)
    raise last_exc


# revision 46
# speedup vs baseline: 1.0023x; 1.0023x over previous
"""Multi-head attention (B=2, S=2048, D=1024, H=16, Dh=64) on 8 TRN2 cores.

Sharding: data-parallel over batch (2) x tensor-parallel over heads (16 -> 4
groups of 4). Core c handles batch c//4, heads [4*(c%4), 4*(c%4)+4).
Each core computes its partial output projection (Wo column slice); the host
sums the 4 per-core partials per batch (the "all-reduce") and adds bo.

All-fp16 data path (fp8/DoubleRow was tried and is numerically dead here:
quantization noise on scores/weights/values does NOT average down — the
attention output is a weighted mean whose magnitude shrinks as fast as the
noise, so fp8 anywhere in the value path lands at ~3-7% output error vs the
2e-2 gate; fp16 gives ~9e-4).

The kernel is PE-bound (fp16 matmul floor ~401k PE cycles = 167us at the
2.4GHz max p-state vs ACT's 133us exp stream). The TRN2 p-state model
punishes recurring PE dependency stalls (they reset the clock ramp and lock
the PE at 1.2GHz), and the cost model serializes all DMA transfers on one
track (~2.9us per 0.5MB s-chunk), which shapes the whole schedule:

  - x streams in s-chunks [P, KD, 512] in exact consumption order on one
    queue; every projection job is runnable right after one chunk lands.
  - V is projected directly in natural [s, j] layout (x chunk stationary,
    Wv moving): no PE transposes of V^T.
  - attn@V(i) runs one full task LATER (task-lag): it reads only buffered
    exp tiles + V, so the PE never stalls on the live exp stream. Only the
    last task consumes its own exps lag-1, and it compresses attn@V(6)
    into its first half so only norm(7) + the qh1 output projection remain
    after the final exp.
  - Filler jobs (jb1/late projections, V s-blocks, qh0 output projection)
    are placed at hand-tuned (task, kb) slots to keep the PE dense.
"""

import numpy as np
from contextlib import ExitStack

import concourse.bass as bass
from concourse import bacc
import concourse.mybir as mybir
import concourse.tile as tile

F32 = mybir.dt.float32
F32R = mybir.dt.float32r
F16 = mybir.dt.float16
AF = mybir.ActivationFunctionType

B = 2
S = 2048
D = 1024
H = 16
DH = 64
NCORES = 8
HL = 4          # heads per core
J = HL * DH     # 256 local projection width
P = 128
KD = D // P     # 8 d-chunks
KB = S // P     # 16 k-blocks
QH = S // 1024  # 2 q-halves of 1024
EB = D // P     # 8 e-blocks
NS = 4          # s-chunks of 512


def build_nc():
    nc = bacc.Bacc()

    xq = nc.dram_tensor("xq", [P, NS, KD, 512], F16, kind="ExternalInput")
    xk = nc.dram_tensor("xk", [P, NS, KD, 512], F16, kind="ExternalInput")
    xv = nc.dram_tensor("xv", [P, NS, KD, 512], F16, kind="ExternalInput")
    wq = nc.dram_tensor("wq", [P, KD, J], F16, kind="ExternalInput")
    wk = nc.dram_tensor("wk", [P, KD, J], F16, kind="ExternalInput")
    wv = nc.dram_tensor("wv", [P, KD, J], F16, kind="ExternalInput")
    wo = nc.dram_tensor("wo", [P, 2, D], F16, kind="ExternalInput")
    out_t = nc.dram_tensor("out_t", [EB, P, S], F16, kind="ExternalOutput")

    with tile.TileContext(nc) as tc, ExitStack() as st:
        const = st.enter_context(tc.tile_pool(name="const", bufs=1))
        persist = st.enter_context(tc.tile_pool(name="persist", bufs=1))
        xpool = st.enter_context(tc.tile_pool(name="xstream", bufs=12))

        wq_sb = const.tile([P, KD, J], F16, tag="wq")
        wk_sb = const.tile([P, KD, J], F16, tag="wk")
        wv_sb = const.tile([P, KD, J], F16, tag="wv")
        wo_sb = const.tile([P, 2, D], F16, tag="wo")

        qt_sb = persist.tile([P, 2, S], F16, tag="qt")   # Q_T [256, 2048]
        kt_sb = persist.tile([P, 2, S], F16, tag="kt")   # K_T
        # V natural layout + per-head ones col: [s_part, kb, h, 65]
        v_sb = persist.tile([P, KB, HL, DH + 1], F16, tag="v")
        ao_sb = persist.tile([P, 2, S], F16, tag="ao")   # normalized attn out ^T

        ones64 = const.tile([1, DH], F32R, tag="ones64")

        # --- DMA: one serial stream; order = consumption order -----------
        xq_t = [xpool.tile([P, KD, 512], F16, tag="xc", name=f"xq{c}")
                for c in range(NS)]
        xk_t = [xpool.tile([P, KD, 512], F16, tag="xc", name=f"xk{c}")
                for c in range(NS)]
        xv_t = [xpool.tile([P, KD, 512], F16, tag="xc", name=f"xv{c}")
                for c in range(NS)]
        ones16 = nc.inline_tensor(np.ones((P, KB * HL), np.float16),
                                  name="ones16")
        ones_f32 = nc.inline_tensor(np.ones((1, DH), np.float32), name="ones_f")
        dma_order = [
            (wk_sb, wk, None),
            (xk_t[0], xk, 0), (wq_sb, wq, None),
            (xq_t[0], xq, 0), (xq_t[1], xq, 1),
            (xk_t[1], xk, 1), (xk_t[2], xk, 2), (xk_t[3], xk, 3),
            (wv_sb, wv, None),
            (xv_t[0], xv, 0), (xv_t[1], xv, 1), (xv_t[2], xv, 2),
            (xv_t[3], xv, 3), (xq_t[2], xq, 2), (xq_t[3], xq, 3),
        ]
        for dst, srcd, sch in dma_order:
            if sch is None:
                nc.sync.dma_start(out=dst[:], in_=srcd[:])
            else:
                nc.sync.dma_start(out=dst[:], in_=srcd[:, sch])
        nc.sync.dma_start(out=v_sb[:, :, :, DH], in_=ones16.ap())
        nc.sync.dma_start(out=ones64[:], in_=ones_f32.ap().bitcast(F32R))
        nc.sync.dma_start(out=wo_sb[:], in_=wo[:])

        # --- attention pipeline with interleaved filler work --------------
        with tc.tile_pool(name="psc", bufs=2, space="PSUM") as psc, tc.tile_pool(
            name="poacc", bufs=2, space="PSUM"
        ) as poacc, tc.tile_pool(name="expp", bufs=17) as expp, tc.tile_pool(
            name="npool", bufs=2
        ) as npool, tc.tile_pool(name="ostage", bufs=4) as opool:

            def proj_job(wsb, xts, dst, sch, jb, nm):
                def f():
                    pp = psc.tile([P, 512], F32, tag="sc",
                                  name=f"pj{nm}{sch}{jb}")
                    for c in range(KD):
                        nc.tensor.matmul(
                            pp[:, :512],
                            wsb[:, c, jb * P:(jb + 1) * P],
                            xts[sch][:, c, :],
                            start=(c == 0),
                            stop=(c == KD - 1),
                        )
                    nc.vector.tensor_copy(
                        dst[:, jb, sch * 512:(sch + 1) * 512], pp[:])
                return f

            def vjob(sb):
                def f():
                    vp = psc.tile([P, J], F32, tag="sc", name=f"vp{sb}")
                    for c in range(KD):
                        nc.tensor.matmul(
                            vp[:, :J],
                            xv_t[sb // 4][:, c, (sb % 4) * P:(sb % 4 + 1) * P],
                            wv_sb[:, c, :],
                            start=(c == 0),
                            stop=(c == KD - 1),
                        )
                    nc.vector.tensor_copy(v_sb[:, sb, :, 0:DH], vp[:])
                return f

            def oproj_eb(qh, eb, evac="dve", tailpool=False,
                         split_dma=False):
                def f():
                    q0 = qh * 1024
                    ob = opool.tile([P, 1024], F16, tag="ob",
                                    name=f"ob{(qh * EB + eb) % 4}")
                    for stl in range(2):
                        s0 = q0 + stl * 512
                        pool = poacc if (tailpool and stl == 1) else psc
                        tg = "oacc" if (tailpool and stl == 1) else "sc"
                        po = pool.tile([P, 512], F32, tag=tg,
                                       name=f"po_{qh}_{eb}_{stl}")
                        for jbx in range(2):
                            nc.tensor.matmul(
                                po[:, :512],
                                wo_sb[:, jbx, eb * P:(eb + 1) * P],
                                ao_sb[:, jbx, s0:s0 + 512],
                                start=(jbx == 0),
                                stop=(jbx == 1),
                            )
                        d = ob[:, stl * 512:(stl + 1) * 512]
                        if evac == "act" or (evac == "mix" and stl == 0):
                            nc.scalar.copy(d, po[:, :512])
                        else:
                            nc.vector.tensor_copy(d, po[:, :512])
                        if split_dma:
                            nc.sync.dma_start(out=out_t[eb][:, s0:s0 + 512],
                                              in_=d)
                    if not split_dma:
                        nc.sync.dma_start(out=out_t[eb][:, q0:q0 + 1024],
                                          in_=ob[:])
                return f

            # filler schedule: fillers[(task, kb)] run inside the kb loop
            # after scores/exp/attn@V; positions tuned against the model
            fillers = {}

            def put(i, kb, fn):
                fillers.setdefault((i, kb), []).append(fn)

            put(0, 3, proj_job(wk_sb, xk_t, kt_sb, 1, 0, "k"))
            put(0, 7, proj_job(wk_sb, xk_t, kt_sb, 2, 0, "k"))
            put(0, 10, proj_job(wk_sb, xk_t, kt_sb, 3, 0, "k"))
            put(0, 0, proj_job(wk_sb, xk_t, kt_sb, 0, 1, "k"))
            put(0, 1, proj_job(wq_sb, xq_t, qt_sb, 0, 1, "q"))
            put(0, 4, proj_job(wq_sb, xq_t, qt_sb, 1, 1, "q"))
            put(0, 5, proj_job(wk_sb, xk_t, kt_sb, 1, 1, "k"))
            put(0, 8, proj_job(wk_sb, xk_t, kt_sb, 2, 1, "k"))
            put(0, 12, proj_job(wk_sb, xk_t, kt_sb, 3, 1, "k"))
            for n in range(3):
                put(0, 13 + n, vjob(n))
            put(1, 0, vjob(3))
            for n in range(4):
                put(1, 1 + n, vjob(4 + n))
            for n in range(4):
                put(1, 5 + n, vjob(8 + n))
            for n in range(4):
                put(1, 9 + n, vjob(12 + n))
            put(2, 3, proj_job(wq_sb, xq_t, qt_sb, 2, 0, "q"))
            put(2, 11, proj_job(wq_sb, xq_t, qt_sb, 3, 0, "q"))
            put(3, 3, proj_job(wq_sb, xq_t, qt_sb, 2, 1, "q"))
            put(3, 11, proj_job(wq_sb, xq_t, qt_sb, 3, 1, "q"))
            # ao(qh0) is complete only after norm(3), which lands at the
            # END of task 4 (task-lag) — oproj(0) fillers start in task 5
            opos = [(5, 2), (5, 7), (5, 12), (6, 2), (6, 7), (6, 12),
                    (7, 2), (7, 7)]
            for eb in range(EB):
                put(*opos[eb], oproj_eb(0, eb))

            def normalize(h, q0, w, oacc, recip, nm):
                def f():
                    jbn = h // 2
                    off = DH * (h % 2)
                    bc = psc.tile([DH, w], F32, tag="sc", name=f"bc{nm}")
                    for n in range(w // 512):
                        nc.tensor.matmul(
                            bc[:, n * 512:(n + 1) * 512],
                            ones64[:],
                            recip[:, n * 512:(n + 1) * 512],
                            start=True,
                            stop=True,
                        )
                    bcast = npool.tile([DH, w], F16, tag="bcast",
                                       name=f"bst{nm}")
                    nc.vector.tensor_copy(bcast[:], bc[:])
                    nc.vector.tensor_mul(
                        ao_sb[off:off + DH, jbn, q0:q0 + w],
                        oacc[0:DH, :],
                        bcast[:],
                    )
                return f

            # pre-task projections: first scores need Q sch0-1 + K sch0 (jb0)
            proj_job(wk_sb, xk_t, kt_sb, 0, 0, "k")()
            proj_job(wq_sb, xq_t, qt_sb, 0, 0, "q")()
            proj_job(wq_sb, xq_t, qt_sb, 1, 0, "q")()

            # attn@V(i) runs one full task later (task-lag): it reads only
            # buffered exps + V, so the PE never stalls on the live exp
            # stream (a per-kb dependency stall would reset the clock ramp
            # and lock the PE at 1.2GHz). The last task additionally
            # consumes its own exps lag-1 and compresses attn@V(6) into
            # its first half so the tail is just norm(7) + the qh1 oproj.
            tasks = [(qh, hh) for qh in range(QH) for hh in range(HL)]
            NT = len(tasks)

            def attn_v(oacc, hh, ex, kb):
                for n in range(2):
                    nc.tensor.matmul(
                        oacc[:, n * 512:(n + 1) * 512],
                        v_sb[:, kb, hh, :],
                        ex[kb][:, n * 512:(n + 1) * 512],
                        start=(kb == 0),
                        stop=(kb == KB - 1),
                    )

            prev = None  # (qh, h, ex) whose attn@V runs this iter
            exn = 0
            for i in range(NT):
                qh, h = tasks[i]
                q0 = qh * 1024
                jb = h // 2
                off = DH * (h % 2)
                cur = (qh, h, {})
                oacc = poacc.tile([DH + 1, 1024], F32, tag="oacc",
                                  name=f"oacc{i % 2}") if prev else None
                oacc7 = poacc.tile([DH + 1, 1024], F32, tag="oacc",
                                   name="oacc7") if i == NT - 1 else None
                for kb in range(KB):
                    sc = psc.tile([P, 1024], F32, tag="sc", name=f"sc{kb % 2}")
                    for n in range(2):
                        nc.tensor.matmul(
                            sc[:, n * 512:(n + 1) * 512],
                            kt_sb[off:off + DH, jb, kb * P:(kb + 1) * P],
                            qt_sb[off:off + DH, jb,
                                  q0 + n * 512:q0 + (n + 1) * 512],
                            start=True,
                            stop=True,
                        )
                    cur[2][kb] = expp.tile([P, 1024], F16, tag="ex",
                                           name=f"ex{exn % 17}")
                    exn += 1
                    nc.scalar.activation(cur[2][kb][:], sc[:], AF.Exp)
                    if prev is not None:
                        if oacc7 is not None:
                            if kb < KB // 2:
                                attn_v(oacc, prev[1], prev[2], 2 * kb)
                                attn_v(oacc, prev[1], prev[2], 2 * kb + 1)
                        else:
                            attn_v(oacc, prev[1], prev[2], kb)
                    if oacc7 is not None and kb >= 1:
                        attn_v(oacc7, h, cur[2], kb - 1)
                    if oacc7 is not None and kb == KB // 2:
                        recip6 = npool.tile([1, 1024], F32R, tag="recip",
                                            name="recip6")
                        with nc.allow_low_precision(reason="softmax denom"):
                            nc.vector.reciprocal(recip6[:],
                                                 oacc[DH:DH + 1, :])
                    if oacc7 is not None and kb == KB // 2 + 2:
                        normalize(prev[1], prev[0] * 1024, 1024, oacc,
                                  recip6, "n6")()
                    for fn in fillers.pop((i, kb), ()):
                        fn()
                if prev is not None and oacc7 is None:
                    recip = npool.tile([1, 1024], F32R, tag="recip",
                                       name=f"recip{i % 2}")
                    with nc.allow_low_precision(reason="fp32r softmax denom"):
                        nc.vector.reciprocal(recip[:], oacc[DH:DH + 1, :])
                    normalize(prev[1], prev[0] * 1024, 1024, oacc, recip,
                              f"n{i % 2}")()
                if oacc7 is not None:
                    attn_v(oacc7, h, cur[2], KB - 1)
                    recip7 = npool.tile([1, 1024], F32R, tag="recip",
                                        name="recip7")
                    with nc.allow_low_precision(reason="fp32r softmax denom"):
                        nc.vector.reciprocal(recip7[:], oacc7[DH:DH + 1, :])
                    normalize(h, q0, 1024, oacc7, recip7, "nd")()
                    prev = None
                else:
                    prev = cur

            # tail: qh1 output projection, po tiles spread over both pools
            for eb in range(EB):
                oproj_eb(1, eb, evac="mix", tailpool=True)()

    nc.finalize()
    return nc


_NC_CACHE = None


def _get_nc():
    global _NC_CACHE
    if _NC_CACHE is None:
        _NC_CACHE = build_nc()
    return _NC_CACHE


def make_in_maps(query, key, value, Wq, Wk, Wv, Wo):
    """Build the 8 per-core input dicts from the full tensors."""
    query = np.asarray(query, np.float32)
    key = np.asarray(key, np.float32)
    value = np.asarray(value, np.float32)
    Wq = np.asarray(Wq, np.float32)
    Wk = np.asarray(Wk, np.float32)
    Wv = np.asarray(Wv, np.float32)
    Wo = np.asarray(Wo, np.float32)

    def pmajor(a2d, inner):  # [Drows, inner] -> [P, Drows//P, inner]
        return np.ascontiguousarray(
            a2d.reshape(KD, P, inner).transpose(1, 0, 2)
        )

    def schunk(a2d):  # X^T [D, S] -> [P, NS, KD, 512]
        return np.ascontiguousarray(
            a2d.reshape(KD, P, NS, 512).transpose(1, 2, 0, 3)
        )

    scale = np.float32(1.0 / np.sqrt(DH))
    xs = {}
    for b in range(B):
        xs[b] = {
            "xq": schunk(np.ascontiguousarray(query[b].T)).astype(np.float16),
            "xk": schunk(np.ascontiguousarray(key[b].T)).astype(np.float16),
            "xv": schunk(np.ascontiguousarray(value[b].T)).astype(np.float16),
        }
    ws = {}
    for hg in range(4):
        sl = slice(hg * J, (hg + 1) * J)
        wo_t = np.ascontiguousarray(Wo[:, sl].T)  # [256, 1024]
        ws[hg] = {
            "wq": pmajor(np.ascontiguousarray(Wq[sl].T * scale), J).astype(
                np.float16),
            "wk": pmajor(np.ascontiguousarray(Wk[sl].T), J).astype(np.float16),
            "wv": pmajor(np.ascontiguousarray(Wv[sl].T), J).astype(np.float16),
            "wo": np.ascontiguousarray(
                wo_t.reshape(2, P, D).transpose(1, 0, 2)
            ).astype(np.float16),
        }
    in_maps = []
    for c in range(NCORES):
        b, hg = c // 4, c % 4
        m = {}
        m.update(xs[b])
        m.update(ws[hg])
        in_maps.append(m)
    return in_maps


def assemble(results, bo):
    """Sum the 4 per-core partials per batch, add bo."""
    bo = np.asarray(bo, np.float32)
    out = np.zeros((B, S, D), np.float32)
    for c in range(NCORES):
        b = c // 4
        part = results[c]["out_t"].astype(np.float32).reshape(D, S).T
        out[b] += part
    out += bo[None, None, :]
    return out


def kernel(query, key, value, Wq, Wk, Wv, Wo, bo):
    import os
    import time

    os.environ.setdefault("NEURON_RT_RESET_CORES", "1")
    from concourse.bass_utils import run_bass_kernel_spmd

    nc = _get_nc()
    in_maps = make_in_maps(query, key, value, Wq, Wk, Wv, Wo)
    last_exc = None
    for attempt in range(3):
        try:
            res = run_bass_kernel_spmd(nc, in_maps, list(range(NCORES)))
            return assemble(res.results, bo)
        except Exception as e:  # transient NRT_EXEC_UNIT_UNRECOVERABLE etc.
            last_exc = e
            time.sleep(2.0)
    raise last_exc
